# revision 23
# baseline (speedup 1.0000x reference)
"""Trainium2 Bass kernel for nn_Loss4PixelReconstruction.

reference: recon = sum_k shift_k(image1) * filters[k]  (11x11 dynamic
per-pixel filter, shared across RGB), loss = mean(sqrt((recon-image2)^2+eps^2)).

Sharding: data-parallel over (N=4) x (H split in 2) -> 8 cores.
Each core: local Charbonnier partial sum; host sums the 8 scalars.

v3 design:
 - Host pre-converts all inputs to bf16 (halves HBM traffic; on-chip
   compute was already bf16). Filters DMA straight into the even/odd
   dx-parity layouts; the 11 dy-shifted image slabs DMA straight from HBM.
 - DVE does 2 batched bf16 multiplies per dy (all even-dx taps in one op,
   all odd-dx taps in another) via overlapping-window access patterns.
   Odd-dx taps are computed in a +1-column-shifted frame (filters stored
   pre-shifted) so every DVE operand keeps 4B alignment for 2x mode.
 - The 121-plane accumulation runs on the otherwise-idle TensorE: identity
   matmuls accumulate each product plane into PSUM fp32 (odd-frame planes
   are read at a +1 column offset, un-shifting them). The identity is
   loaded once with a standalone ldweights; the matmuls set ldweights=False
   so the PE never reloads weights.
"""

import sys

sys.path.insert(0, "/opt/trn_rl_repo")

import numpy as np
import ml_dtypes

BF16 = ml_dtypes.bfloat16

K = 11
PAD = 5
EPS = 1e-3
N, C, H, W = 4, 3, 256, 256
HSH = 128               # output rows per core
IMG_H = HSH + 2 * PAD   # 138 padded input rows per core
W_PAD = 268             # padded input cols (5 + 256 + 7)
CW = C * W              # 768
WO = 258                # odd-frame product width (W + 2)

_CACHE = {}
LAST_RESULTS = None
NO_LDW = True  # use ldweights=False matmuls (identity loaded once)
GPS_DYS = ()  # gpsimd tensor_tensor measured ~4x slower than DVE; unused


def _mm_noldw(nc, mybir, out, lhsT, rhs, start, stop):
    """matmul that reuses the already-loaded stationary operand
    (InstMatmult with ldweights=False; weights stay in ins for dep
    tracking and for CoreSim, which reads them per-instruction)."""
    if not NO_LDW:
        return nc.tensor.matmul(out=out, lhsT=lhsT, rhs=rhs,
                                start=start, stop=stop)
    eng = nc.tensor
    ifmap_ap = eng.lower_ap(rhs.opt({0}), opt=False)
    weights_ap = eng.lower_ap(lhsT.opt({0}), opt=False, for_matmul_weights=True)
    out_ap = eng.lower_ap(out)
    return eng.add_instruction(
        mybir.InstMatmult(
            name=eng.bass.get_next_instruction_name(),
            replication_resolution=0,
            replication_shift_amnt=0,
            replication_num_rows=0,
            start_tensor_calc=start,
            stop_tensor_calc=stop,
            ins=[ifmap_ap, weights_ap],
            outs=[out_ap],
            tile_position=(0, 0),
            tile_size=(128, 128),
            ldweights=False,
        )
    )


def _build_nc():
    import concourse.tile as tile
    from concourse import bacc, mybir
    from concourse import bass_isa
    import concourse.bass as bass
    from concourse.masks import make_identity
    from contextlib import ExitStack

    bf16 = mybir.dt.bfloat16
    f32 = mybir.dt.float32
    MUL = mybir.AluOpType.mult
    SUB = mybir.AluOpType.subtract
    AP = bass.AP

    nc = bacc.Bacc("TRN2", target_bir_lowering=False, debug=False)

    img1p = nc.declare_dram_parameter("img1p", [C, IMG_H, W_PAD], bf16, isOutput=False)
    img2 = nc.declare_dram_parameter("img2", [C, HSH, W], bf16, isOutput=False)
    flt = nc.declare_dram_parameter("flt", [K * K, HSH, W], bf16, isOutput=False)
    out = nc.declare_dram_parameter("out", [1, 1], f32, isOutput=True)

    with ExitStack() as ctx:
        tc = ctx.enter_context(tile.TileContext(nc))
        imp = ctx.enter_context(tc.tile_pool(name="im", bufs=1))
        fbp = ctx.enter_context(tc.tile_pool(name="fb", bufs=2))
        prp = ctx.enter_context(tc.tile_pool(name="pr", bufs=2))
        psp = ctx.enter_context(tc.tile_pool(name="ps", bufs=1, space="PSUM"))
        tlp = ctx.enter_context(tc.tile_pool(name="tl", bufs=1))

        ident = imp.tile([HSH, HSH], bf16)
        make_identity(nc, ident[:])
        nc.tensor.ldweights(weights=ident[:])
        ones = tlp.tile([HSH, 1], f32)
        nc.gpsimd.memset(ones[:], 1.0)

        # imall[:, dy, :, :] = bf16 image rows (dy .. dy+127) of the padded slab
        imall = imp.tile([HSH, K, C, W_PAD], bf16)

        # PSUM accumulator for recon (c,w) flattened: [0:512] + [512:768]
        accA = psp.tile([HSH, 512], f32)
        accB = psp.tile([HSH, CW - 512], f32)

        im_t = imall[:].tensor
        im_par = K * C * W_PAD  # partition stride of imall (elements)

        i2b = tlp.tile([HSH, C, W], bf16)

        for dy in range(K):
            nc.sync.dma_start(
                imall[:, dy, :, :],
                img1p[:, dy:dy + HSH, :].rearrange("c h w -> h c w"),
            )
            if dy == 2:
                nc.sync.dma_start(
                    i2b[:], img2[:, :, :].rearrange("c h w -> h c w")
                )

            # bf16 filters: fe = even-dx planes; fo = odd-dx planes shifted
            # +1 col (cols 0 and 257 are never consumed downstream)
            fe = fbp.tile([HSH, 6, W], bf16, tag="fe")
            fo = fbp.tile([HSH, 5, WO], bf16, tag="fo")
            if dy == 0:
                # split the first filter DMA so the first multiply can
                # start as soon as the first 3 planes land
                nc.sync.dma_start(
                    fe[:, 0:3, :],
                    flt[0:6:2, :, :].rearrange("k h w -> h k w"),
                )
                nc.sync.dma_start(
                    fe[:, 3:6, :],
                    flt[6:K:2, :, :].rearrange("k h w -> h k w"),
                )
            else:
                nc.sync.dma_start(
                    fe[:],
                    flt[dy * K:dy * K + K:2, :, :].rearrange("k h w -> h k w"),
                )
            nc.gpsimd.memset(fo[:, :, 0:1], 0.0)
            nc.gpsimd.memset(fo[:, :, WO - 1:WO], 0.0)
            nc.sync.dma_start(
                fo[:, :, 1:W + 1],
                flt[dy * K + 1:dy * K + K:2, :, :].rearrange("k h w -> h k w"),
            )

            # products: pe[p,j,c,u] = im[p, c, u+2j] * fe[p,j,u]
            #           po[p,j,c,u] = im[p, c, u+2j] * fo[p,j,u]
            #           (po holds output w = u-1; filters were pre-shifted)
            pe = prp.tile([HSH, 6, C, W], bf16, tag="pe")
            po = prp.tile([HSH, 5, C, WO], bf16, tag="po")
            base = dy * C * W_PAD
            in0e = AP(im_t, base, [[im_par, HSH], [2, 6], [W_PAD, C], [1, W]])
            in1e = AP(fe[:].tensor, fe[:].offset,
                      [[6 * W, HSH], [W, 6], [0, C], [1, W]])
            in0o = AP(im_t, base, [[im_par, HSH], [2, 5], [W_PAD, C], [1, WO]])
            in1o = AP(fo[:].tensor, fo[:].offset,
                      [[5 * WO, HSH], [WO, 5], [0, C], [1, WO]])

            # For the last dy, split each multiply into halves so the PE can
            # start draining the final accumulations sooner.
            # One LDWEIGHTS per product group: matmul waits that exceed the
            # 1-per-instruction limit get migrated to the most recent
            # ldweights by move_matmul_waits_to_ldweights, so it must sit
            # adjacent.
            esplits = ((0, 3), (3, 6)) if dy == 0 else ((0, 6),)
            for jl, jh in esplits:
                sub = AP(in0e.tensor, in0e.offset + 2 * jl,
                         [[im_par, HSH], [2, jh - jl], [W_PAD, C], [1, W]])
                fsub = AP(in1e.tensor, in1e.offset + W * jl,
                          [[6 * W, HSH], [W, jh - jl], [0, C], [1, W]])
                nc.vector.tensor_tensor(pe[:, jl:jh, :, :], sub, fsub, MUL)
                if NO_LDW:
                    nc.tensor.ldweights(weights=ident[:])
                for j in range(jl, jh):
                    first = (dy == 0) and (j == 0)
                    _mm_noldw(nc, mybir, accA[:], ident[:], pe[:, j, 0:2, :],
                              first, False)
                    _mm_noldw(nc, mybir, accB[:], ident[:], pe[:, j, 2, :],
                              first, False)
            osplits = ((0, 5),)
            for jl, jh in osplits:
                sub = AP(in0o.tensor, in0o.offset + 2 * jl,
                         [[im_par, HSH], [2, jh - jl], [W_PAD, C], [1, WO]])
                fsub = AP(in1o.tensor, in1o.offset + WO * jl,
                          [[5 * WO, HSH], [WO, jh - jl], [0, C], [1, WO]])
                nc.vector.tensor_tensor(po[:, jl:jh, :, :], sub, fsub, MUL)
                if NO_LDW:
                    nc.tensor.ldweights(weights=ident[:])
                for j in range(jl, jh):
                    last = (dy == K - 1) and (j == 4)
                    _mm_noldw(nc, mybir, accA[:], ident[:],
                              po[:, j, 0:2, 1:W + 1], False, last)
                    _mm_noldw(nc, mybir, accB[:], ident[:],
                              po[:, j, 2, 1:W + 1], False, last)

        # Charbonnier tail
        i2f = i2b[:].rearrange("p c w -> p (c w)")
        diff = tlp.tile([HSH, CW], bf16)
        nc.vector.tensor_tensor(diff[:, 0:512], accA[:], i2f[:, 0:512], SUB)
        nc.vector.tensor_tensor(diff[:, 512:CW], accB[:], i2f[:, 512:CW], SUB)
        d2 = tlp.tile([HSH, CW], bf16)
        nc.vector.tensor_tensor(d2[:], diff[:], diff[:], MUL)
        charb = tlp.tile([HSH, CW], bf16)
        rowsum = tlp.tile([HSH, 1], f32)
        eps2 = tlp.tile([HSH, 1], f32)
        nc.vector.memset(eps2[:], EPS * EPS)
        nc.scalar.activation(
            charb[:], d2[:], mybir.ActivationFunctionType.Sqrt,
            bias=eps2[:], scale=1.0, accum_out=rowsum[:],
        )
        # partition reduction via ones-weights matmul (self-loading fp32
        # path; runs after all identity matmuls on the in-order PE queue)
        tot_ps = psp.tile([1, 1], f32)
        nc.tensor.matmul(out=tot_ps[:], lhsT=ones[:], rhs=rowsum[:],
                         start=True, stop=True)
        total = tlp.tile([1, 1], f32)
        nc.scalar.copy(total[:], tot_ps[:])
        nc.sync.dma_start(out[:, :], total[:, :])

    nc.compile()
    return nc


def _get_nc():
    if "nc" not in _CACHE:
        _CACHE["nc"] = _build_nc()
    return _CACHE["nc"]


def _shard_inputs(image1, image2, filters):
    image1 = np.asarray(image1, np.float32).astype(BF16)
    image2 = np.asarray(image2, np.float32).astype(BF16)
    filters = np.asarray(filters, np.float32).astype(BF16)
    in_maps = []
    for core in range(8):
        n, hb = core // 2, core % 2
        h0 = hb * HSH
        img1p = np.zeros((C, IMG_H, W_PAD), BF16)
        lo = max(0, h0 - PAD)
        hi = min(H, h0 + HSH + PAD)
        img1p[:, lo - (h0 - PAD):lo - (h0 - PAD) + (hi - lo), PAD:PAD + W] = \
            image1[n, :, lo:hi, :]
        in_maps.append({
            "img1p": img1p,
            "img2": np.ascontiguousarray(image2[n, :, h0:h0 + HSH, :]),
            "flt": np.ascontiguousarray(filters[n, :, h0:h0 + HSH, :]),
        })
    return in_maps


def kernel(image1, image2, filters):
    global LAST_RESULTS
    import os
    from concourse.bass_utils import run_bass_kernel_spmd

    nc = _get_nc()
    in_maps = _shard_inputs(image1, image2, filters)
    trace = bool(int(os.environ.get("KERNEL_TRACE", "0")))
    res = run_bass_kernel_spmd(nc, in_maps, list(range(8)), trace=trace)
    LAST_RESULTS = res
    parts = [float(res.results[i]["out"][0, 0]) for i in range(8)]
    return np.float32(sum(parts) / (N * C * H * W))
